# revision 9
# baseline (speedup 1.0000x reference)
"""Trainium2 Bass kernel for nn_GateCircuit (14-qubit batched gate circuit).

Math: the reference applies RX(x@W.T[:,i]) then RY(params[i]) on wire i of
|0...0> (a product state stays a product state since each gate hits a distinct
wire), then a CNOT ladder CNOT(i, i+1), then measures <Z_0>.  Qubit 0 is only
ever a CNOT *control*, so its marginal is untouched by the ladder; the
expectation collapses to the single-qubit value

    <Z_0> = cos(x @ W[0]) * cos(params[0])
    out   = sigmoid(<Z_0>)

Sharding: pure data parallel, batch 4096 split 512 per core across 8 cores;
W row 0 and params[0] shipped as one [1,257] row (1KB DMA) and broadcast
across partitions on-chip by the PE (outer product with a ones row into
PSUM) -- no 128KB host-broadcast DMA, no gpsimd ucode library.

On-device per core (all f32):
  x lands as one [128, 1024] tile (partition p = samples 4p..4p+3), split as
  four 128KB DMAs with 1KB lines, two per HWDGE ring, each with its own
  completion semaphore so dot n starts as block n lands.
  z[:, n] = sum_f (x*inv2pi) * w            4x DVE STT with accumulator, w
                                            read straight from PSUM; 1/2pi
                                            folded in, z in whole periods
  k = int(z)                                f32->i32 cast rounds to nearest
                                            on HW (verified on device)
  f = k - z in [-0.5, 0.5]                  one STT; sign dies in v = f^2
  P(v) = C0+C1 v+C2 v^2+C3 v^3 ~= cos(2pi f) = cos(x@W[0]), err 1.4e-3
  out = sigmoid(Pp*q3 + Pp*C0)              one ACT op; per-partition AP
      = sigmoid(cos(p0)*cos(x@W[0]))        scale/bias, Pp = P(v_p) = cos(p0)
                                            from the same chain on gpsimd
                                            (native TS/TT/copy ops only)
A dummy activation right after the scalar engine's DMA injects pulls both
act-table loads off the critical path (they'd otherwise run right before
the final sigmoid).  Output store goes out on the scalar ring, injected by
the engine that just produced the sigmoid.
"""

import math

import numpy as np

_NCORES = 8
_B = 4096
_F = 256
_BS = _B // _NCORES  # 512 samples per core
_NT = _BS // 128     # 4 sample-blocks per partition
_INV_TWO_PI = float(1.0 / (2.0 * math.pi))

# P(v) = C0 + C1 v + C2 v^2 + C3 v^3 ~= cos(2pi f), v = f^2, f in [-.5, .5]
_C0 = 0.9985678609910458
_C1 = -19.552759014070162
_C2 = 61.10740166704636
_C3 = -59.580321884808846

_CACHE: dict = {}


def _build():
    import concourse.bacc as bacc
    import concourse.mybir as mybir
    import concourse.tile as tile

    f32 = mybir.dt.float32
    i32 = mybir.dt.int32
    Alu = mybir.AluOpType
    Act = mybir.ActivationFunctionType

    nc = bacc.Bacc("TRN2", target_bir_lowering=False, debug=False,
                   num_devices=_NCORES)

    x_d = nc.dram_tensor("x", [_BS, _F], f32, kind="ExternalInput")
    wp_d = nc.dram_tensor("wp", [1, _F + 1], f32, kind="ExternalInput")
    o_d = nc.dram_tensor("o", [_BS], f32, kind="ExternalOutput")

    with tile.TileContext(nc) as tc:
        with (
            tc.tile_pool(name="xin", bufs=1) as xpool,
            tc.tile_pool(name="scratch", bufs=2) as spool,
            tc.tile_pool(name="small", bufs=1) as zpool,
            tc.tile_pool(name="psum", bufs=1, space="PSUM") as ppool,
        ):
            # --- ones row for the PE broadcast (no data deps) ---
            ones = zpool.tile([1, 128], f32)
            nc.gpsimd.memset(ones[:], 1.0)
            dummy = zpool.tile([1, 1], f32)

            # --- input DMAs.  wp first on the sync ring (tiny, and
            # everything depends on it; the scalar engine is kept free of
            # input DMAs so its act-table loads stall nothing).  x halves:
            # one on the sync HWDGE ring, one on the gpsimd SWDGE ring,
            # 2KB lines each. ---
            wp_row = zpool.tile([1, _F + 1], f32)
            nc.sync.dma_start(wp_row[:], wp_d[:, :])
            xr = x_d.ap().rearrange("(p n) f -> p (n f)", n=_NT)  # [128,1024]
            xt = xpool.tile([128, _NT * _F], f32)
            half = _NT * _F // 2
            nc.sync.dma_start(xt[:, 0:half], xr[:, 0:half])
            nc.gpsimd.dma_start(xt[:, half:], xr[:, half:])

            # dummy activation: forces the act-table loads to be emitted
            # early on the (otherwise idle) scalar engine, not in front of
            # the final sigmoid
            nc.scalar.activation(dummy[:], ones[0:1, 0:1], Act.Sigmoid)

            # --- PE broadcast: wb_ps[p, j] = ones[p] * wp_row[j].
            # A dep-free warmup matmul first lifts the PE out of its low
            # p-state and absorbs first-instruction overhead. ---
            bank = ppool.tile([128, 512], f32)
            warm = ppool.tile([128, 512], f32)
            nc.tensor.matmul(warm[:, 0:128], ones[:], ones[:],
                             start=True, stop=True)
            wb_ps = bank[:, 0:_F + 1]
            nc.tensor.matmul(wb_ps, ones[:], wp_row[:], start=True, stop=True)

            # --- params chain on gpsimd: Pp = P(frac(p0/2pi)^2) = cos(p0),
            #     Pb = Pp*C0.  [128,1] native ops; gpsimd can't read PSUM,
            #     so DVE first copies the p0 column into SBUF. ---
            p0 = zpool.tile([128, 1], f32)
            nc.vector.tensor_copy(p0[:], wb_ps[:, _F:_F + 1])
            pz = zpool.tile([128, 1], f32)
            nc.gpsimd.tensor_scalar_mul(pz[:], p0[:], _INV_TWO_PI)
            pk = zpool.tile([128, 1], i32)
            nc.gpsimd.tensor_copy(pk[:], pz[:])
            pkf = zpool.tile([128, 1], f32)
            nc.gpsimd.tensor_copy(pkf[:], pk[:])
            pd = zpool.tile([128, 1], f32)
            nc.gpsimd.tensor_tensor(pd[:], pz[:], pkf[:], op=Alu.subtract)
            pv = zpool.tile([128, 1], f32)
            nc.gpsimd.tensor_tensor(pv[:], pd[:], pd[:], op=Alu.mult)
            ps1 = zpool.tile([128, 1], f32)
            nc.gpsimd.tensor_scalar(ps1[:], pv[:], _C3, _C2,
                                    op0=Alu.mult, op1=Alu.add)
            pm1 = zpool.tile([128, 1], f32)
            nc.gpsimd.tensor_tensor(pm1[:], ps1[:], pv[:], op=Alu.mult)
            ps2 = zpool.tile([128, 1], f32)
            nc.gpsimd.tensor_scalar(ps2[:], pm1[:], _C1, 1.0,
                                    op0=Alu.add, op1=Alu.mult)
            ps3 = zpool.tile([128, 1], f32)
            nc.gpsimd.tensor_tensor(ps3[:], ps2[:], pv[:], op=Alu.mult)
            pp = zpool.tile([128, 1], f32)
            nc.gpsimd.tensor_scalar(pp[:], ps3[:], _C0, 1.0,
                                    op0=Alu.add, op1=Alu.mult)
            pb = zpool.tile([128, 1], f32)
            nc.gpsimd.tensor_scalar(pb[:], ps3[:], _C0, _C0,
                                    op0=Alu.add, op1=Alu.mult)

            # --- dot products z[:, n] = sum_f x_blk_n*inv2pi * w  (DVE) ---
            w256 = wb_ps[:, 0:_F]
            z = zpool.tile([128, _NT], f32)
            for n in range(_NT):
                prod = spool.tile([128, _F], f32)
                nc.vector.scalar_tensor_tensor(
                    prod[:], xt[:, n * _F:(n + 1) * _F], _INV_TWO_PI, w256,
                    op0=Alu.mult, op1=Alu.mult,
                    accum_out=z[:, n:n + 1],
                )

            # --- range reduce + cos poly (DVE): q3 = P(v) - C0 ---
            k = zpool.tile([128, _NT], i32)
            nc.vector.tensor_copy(k[:], z[:])
            kf = zpool.tile([128, _NT], f32)
            nc.vector.tensor_copy(kf[:], k[:])
            f = zpool.tile([128, _NT], f32)
            nc.vector.scalar_tensor_tensor(f[:], kf[:], 0.0, z[:],
                                           op0=Alu.bypass, op1=Alu.subtract)
            v = zpool.tile([128, _NT], f32)
            nc.vector.tensor_tensor(v[:], f[:], f[:], op=Alu.mult)
            q1 = zpool.tile([128, _NT], f32)
            nc.vector.tensor_scalar(q1[:], v[:], _C3, _C2,
                                    op0=Alu.mult, op1=Alu.add)
            q2 = zpool.tile([128, _NT], f32)
            nc.vector.scalar_tensor_tensor(q2[:], q1[:], 0.0, v[:],
                                           op0=Alu.bypass, op1=Alu.mult)
            q3 = zpool.tile([128, _NT], f32)
            nc.vector.scalar_tensor_tensor(q3[:], q2[:], _C1, v[:],
                                           op0=Alu.add, op1=Alu.mult)

            # --- out = sigmoid(Pp*q3 + Pb) = sigmoid(cos(p0)cos(x@W0)) ---
            ot = zpool.tile([128, _NT], f32)
            nc.scalar.activation(ot[:], q3[:], Act.Sigmoid,
                                 bias=pb[:, :], scale=pp[:, :])

            nc.scalar.dma_start(o_d.ap().rearrange("(p n) -> p n", n=_NT),
                                ot[:])

    nc.compile()
    return nc


def _get_nc():
    if "nc" not in _CACHE:
        _CACHE["nc"] = _build()
    return _CACHE["nc"]


def _in_maps(x, W, params):
    x = np.ascontiguousarray(np.asarray(x, dtype=np.float32))
    W = np.asarray(W, dtype=np.float32)
    params = np.asarray(params, dtype=np.float32)
    wp = np.concatenate([W[0], params[0:1]]).astype(np.float32)[None, :]
    wp = np.ascontiguousarray(wp)
    return [
        {"x": x[c * _BS:(c + 1) * _BS], "wp": wp}
        for c in range(_NCORES)
    ]


def run_spmd(x, W, params, **kw):
    """Compile (cached) and run on 8 cores; returns BassKernelResults.

    Retries a few times: the axon-relayed device occasionally reports a
    transient NRT_EXEC_UNIT_UNRECOVERABLE that clears on the next attempt.
    """
    import time

    from concourse import bass_utils

    nc = _get_nc()
    in_maps = _in_maps(x, W, params)
    last = None
    for attempt in range(4):
        try:
            return bass_utils.run_bass_kernel_spmd(
                nc, in_maps, list(range(_NCORES)), **kw
            )
        except Exception as e:  # transient device/relay errors
            last = e
            time.sleep(2.0 * (attempt + 1))
    raise last


def kernel(x, W, params):
    res = run_spmd(x, W, params)
    return np.concatenate([res.results[c]["o"] for c in range(_NCORES)], axis=0)


# revision 11
# speedup vs baseline: 1.1073x; 1.1073x over previous
"""Trainium2 Bass kernel for nn_GateCircuit (14-qubit batched gate circuit).

Math: the reference applies RX(x@W.T[:,i]) then RY(params[i]) on wire i of
|0...0> (a product state stays a product state since each gate hits a distinct
wire), then a CNOT ladder CNOT(i, i+1), then measures <Z_0>.  Qubit 0 is only
ever a CNOT *control*, so its marginal is untouched by the ladder; the
expectation collapses to the single-qubit value

    <Z_0> = cos(x @ W[0]) * cos(params[0])
    out   = sigmoid(<Z_0>)

Sharding: pure data parallel, batch 4096 split 512 per core across 8 cores;
W row 0 and params[0] shipped as one [1,257] row and replicated across the
128 SBUF partitions by a partition-broadcast DMA (0-stride source AP).

No activation engine at all: sigmoid is a degree-2 odd polynomial on DVE
(err 1.3e-5), so there are no act-table loads -- the scalar engine runs
only DMA injects and its HWDGE ring streams at full rate.

Ring budget (HWDGE rings expand ~1 descriptor per ~10ns; every [128,*]
transfer costs 128 descriptors, so each ring gets at most two of them):
  scalar ring:  wp broadcast (first), output store partitions 0..63
  sync ring:    x half A [128 x 2KB lines], output store partitions 64..127
  gpsimd SWDGE: x half B [128 x 2KB lines] (injected first on gpsimd)

On-device per core (all f32):
  z[:, n] = sum_f (x*inv2pi) * w            4x DVE STT with accumulator;
                                            1/2pi folded in, z in periods
  k = int(z)                                f32->i32 cast rounds to nearest
                                            on HW (verified on device)
  f = k - z in [-0.5, 0.5]                  one STT; sign dies in v = f^2
  P(v) = C0+C1 v+C2 v^2+C3 v^3 ~= cos(2pi f) = cos(x@W[0]), err 1.4e-3
  a = Pp*q3 + Pb = cos(p0)*cos(x@W[0])      Pp = P(v_p) = cos(p0), Pb=Pp*C0,
                                            computed on gpsimd off-path
  out = 0.5 + a*(E0 + E1 u + E2 u^2), u=a^2 degree-2 odd sigmoid on DVE
"""

import math

import numpy as np

_NCORES = 8
_B = 4096
_F = 256
_BS = _B // _NCORES  # 512 samples per core
_NT = _BS // 128     # 4 sample-blocks per partition
_INV_TWO_PI = float(1.0 / (2.0 * math.pi))

# P(v) = C0 + C1 v + C2 v^2 + C3 v^3 ~= cos(2pi f), v = f^2, f in [-.5, .5]
_C0 = 0.9985678609910458
_C1 = -19.552759014070162
_C2 = 61.10740166704636
_C3 = -59.580321884808846
# sigmoid(a) = 0.5 + a*(E0 + E1 u + E2 u^2), u = a^2, a in [-1.01, 1.01]
_E0 = 0.24999587
_E1 = -0.02074685
_E2 = 0.00181964

_CACHE: dict = {}


def _build():
    import concourse.bacc as bacc
    import concourse.mybir as mybir
    import concourse.tile as tile

    f32 = mybir.dt.float32
    i32 = mybir.dt.int32
    Alu = mybir.AluOpType

    nc = bacc.Bacc("TRN2", target_bir_lowering=False, debug=False,
                   num_devices=_NCORES)

    x_d = nc.dram_tensor("x", [_BS, _F], f32, kind="ExternalInput")
    wp_d = nc.dram_tensor("wp", [1, _F + 1], f32, kind="ExternalInput")
    o_d = nc.dram_tensor("o", [_BS], f32, kind="ExternalOutput")

    with tile.TileContext(nc) as tc:
        with (
            tc.tile_pool(name="xin", bufs=1) as xpool,
            tc.tile_pool(name="scratch", bufs=2) as spool,
            tc.tile_pool(name="small", bufs=1) as zpool,
        ):
            # --- input DMAs ---
            wb = zpool.tile([128, _F + 1], f32)
            nc.scalar.dma_start(wb[:], wp_d.ap().partition_broadcast(128))
            xr = x_d.ap().rearrange("(p n) f -> p (n f)", n=_NT)  # [128,1024]
            xt = xpool.tile([128, _NT * _F], f32)
            half = _NT * _F // 2
            nc.gpsimd.dma_start(xt[:, half:], xr[:, half:])
            nc.sync.dma_start(xt[:, 0:half], xr[:, 0:half])

            # --- params chain on gpsimd: Pp = P(frac(p0/2pi)^2) = cos(p0),
            #     Pb = Pp*C0.  [128,1] native ops, off the DVE path. ---
            pz = zpool.tile([128, 1], f32)
            nc.gpsimd.tensor_scalar_mul(pz[:], wb[:, _F:_F + 1], _INV_TWO_PI)
            pk = zpool.tile([128, 1], i32)
            nc.gpsimd.tensor_copy(pk[:], pz[:])
            pkf = zpool.tile([128, 1], f32)
            nc.gpsimd.tensor_copy(pkf[:], pk[:])
            pd = zpool.tile([128, 1], f32)
            nc.gpsimd.tensor_tensor(pd[:], pz[:], pkf[:], op=Alu.subtract)
            pv = zpool.tile([128, 1], f32)
            nc.gpsimd.tensor_tensor(pv[:], pd[:], pd[:], op=Alu.mult)
            ps1 = zpool.tile([128, 1], f32)
            nc.gpsimd.tensor_scalar(ps1[:], pv[:], _C3, _C2,
                                    op0=Alu.mult, op1=Alu.add)
            pm1 = zpool.tile([128, 1], f32)
            nc.gpsimd.tensor_tensor(pm1[:], ps1[:], pv[:], op=Alu.mult)
            ps2 = zpool.tile([128, 1], f32)
            nc.gpsimd.tensor_scalar(ps2[:], pm1[:], _C1, 1.0,
                                    op0=Alu.add, op1=Alu.mult)
            ps3 = zpool.tile([128, 1], f32)
            nc.gpsimd.tensor_tensor(ps3[:], ps2[:], pv[:], op=Alu.mult)
            pp = zpool.tile([128, 1], f32)
            nc.gpsimd.tensor_scalar(pp[:], ps3[:], _C0, 1.0,
                                    op0=Alu.add, op1=Alu.mult)
            pb = zpool.tile([128, 1], f32)
            nc.gpsimd.tensor_scalar(pb[:], ps3[:], _C0, _C0,
                                    op0=Alu.add, op1=Alu.mult)

            # --- dot products z[:, n] = sum_f x_blk_n*inv2pi * w  (DVE) ---
            w256 = wb[:, 0:_F]
            z = zpool.tile([128, _NT], f32)
            for n in range(_NT):
                prod = spool.tile([128, _F], f32)
                nc.vector.scalar_tensor_tensor(
                    prod[:], xt[:, n * _F:(n + 1) * _F], _INV_TWO_PI, w256,
                    op0=Alu.mult, op1=Alu.mult,
                    accum_out=z[:, n:n + 1],
                )

            # --- range reduce + cos poly (DVE): q3 = P(v) - C0 ---
            k = zpool.tile([128, _NT], i32)
            nc.vector.tensor_copy(k[:], z[:])
            kf = zpool.tile([128, _NT], f32)
            nc.vector.tensor_copy(kf[:], k[:])
            f = zpool.tile([128, _NT], f32)
            nc.vector.scalar_tensor_tensor(f[:], kf[:], 0.0, z[:],
                                           op0=Alu.bypass, op1=Alu.subtract)
            v = zpool.tile([128, _NT], f32)
            nc.vector.tensor_tensor(v[:], f[:], f[:], op=Alu.mult)
            q1 = zpool.tile([128, _NT], f32)
            nc.vector.tensor_scalar(q1[:], v[:], _C3, _C2,
                                    op0=Alu.mult, op1=Alu.add)
            q2 = zpool.tile([128, _NT], f32)
            nc.vector.scalar_tensor_tensor(q2[:], q1[:], 0.0, v[:],
                                           op0=Alu.bypass, op1=Alu.mult)
            q3 = zpool.tile([128, _NT], f32)
            nc.vector.scalar_tensor_tensor(q3[:], q2[:], _C1, v[:],
                                           op0=Alu.add, op1=Alu.mult)

            # --- a = Pp*q3 + Pb;  out = 0.5 + a*(E0 + E1 u + E2 u^2) ---
            a = zpool.tile([128, _NT], f32)
            nc.vector.tensor_scalar(a[:], q3[:], pp[:, :], pb[:, :],
                                    op0=Alu.mult, op1=Alu.add)
            u = zpool.tile([128, _NT], f32)
            nc.vector.tensor_tensor(u[:], a[:], a[:], op=Alu.mult)
            h1 = zpool.tile([128, _NT], f32)
            nc.vector.tensor_scalar(h1[:], u[:], _E2, _E1,
                                    op0=Alu.mult, op1=Alu.add)
            h2 = zpool.tile([128, _NT], f32)
            nc.vector.scalar_tensor_tensor(h2[:], h1[:], 0.0, u[:],
                                           op0=Alu.bypass, op1=Alu.mult)
            h3 = zpool.tile([128, _NT], f32)
            nc.vector.scalar_tensor_tensor(h3[:], h2[:], _E0, a[:],
                                           op0=Alu.add, op1=Alu.mult)
            ot = zpool.tile([128, _NT], f32)
            nc.vector.tensor_scalar(ot[:], h3[:], 1.0, 0.5,
                                    op0=Alu.mult, op1=Alu.add)

            # --- output store, split across the two HWDGE rings ---
            orr = o_d.ap().rearrange("(p n) -> p n", n=_NT)
            nc.scalar.dma_start(orr[0:64], ot[0:64, :])
            nc.sync.dma_start(orr[64:128], ot[64:128, :])

    nc.compile()
    return nc


def _get_nc():
    if "nc" not in _CACHE:
        _CACHE["nc"] = _build()
    return _CACHE["nc"]


def _in_maps(x, W, params):
    x = np.ascontiguousarray(np.asarray(x, dtype=np.float32))
    W = np.asarray(W, dtype=np.float32)
    params = np.asarray(params, dtype=np.float32)
    wp = np.concatenate([W[0], params[0:1]]).astype(np.float32)[None, :]
    wp = np.ascontiguousarray(wp)
    return [
        {"x": x[c * _BS:(c + 1) * _BS], "wp": wp}
        for c in range(_NCORES)
    ]


def run_spmd(x, W, params, **kw):
    """Compile (cached) and run on 8 cores; returns BassKernelResults.

    Retries a few times: the axon-relayed device occasionally reports a
    transient NRT_EXEC_UNIT_UNRECOVERABLE that clears on the next attempt.
    """
    import time

    from concourse import bass_utils

    nc = _get_nc()
    in_maps = _in_maps(x, W, params)
    last = None
    for attempt in range(4):
        try:
            return bass_utils.run_bass_kernel_spmd(
                nc, in_maps, list(range(_NCORES)), **kw
            )
        except Exception as e:  # transient device/relay errors
            last = e
            time.sleep(2.0 * (attempt + 1))
    raise last


def kernel(x, W, params):
    res = run_spmd(x, W, params)
    return np.concatenate([res.results[c]["o"] for c in range(_NCORES)], axis=0)


# revision 12
# speedup vs baseline: 1.1233x; 1.0144x over previous
"""Trainium2 Bass kernel for nn_GateCircuit (14-qubit batched gate circuit).

Math: the reference applies RX(x@W.T[:,i]) then RY(params[i]) on wire i of
|0...0> (a product state stays a product state since each gate hits a distinct
wire), then a CNOT ladder CNOT(i, i+1), then measures <Z_0>.  Qubit 0 is only
ever a CNOT *control*, so its marginal is untouched by the ladder; the
expectation collapses to the single-qubit value

    <Z_0> = cos(x @ W[0]) * cos(params[0])
    out   = sigmoid(<Z_0>)

Sharding: pure data parallel, batch 4096 split 512 per core across 8 cores;
W row 0 and params[0] shipped as one [1,257] row and replicated across the
128 SBUF partitions by a partition-broadcast DMA (0-stride source AP).

No activation engine at all: sigmoid is a degree-2 odd polynomial on DVE
(err 1.3e-5), so there are no act-table loads -- the scalar engine runs
only DMA injects and its HWDGE ring streams at full rate.

Ring budget (HWDGE rings expand ~1 descriptor per ~10ns; every [128,*]
transfer costs 128 descriptors, so each ring gets at most two of them):
  scalar ring:  wp broadcast (first), output store partitions 0..63
  sync ring:    x half A [128 x 2KB lines], output store partitions 64..127
  gpsimd SWDGE: x half B [128 x 2KB lines] (injected first on gpsimd)

On-device per core (all f32):
  z[:, n] = sum_f (x*inv2pi) * w            4x DVE STT with accumulator;
                                            1/2pi folded in, z in periods
  k = int(z)                                f32->i32 cast rounds to nearest
                                            on HW (verified on device)
  f = k - z in [-0.5, 0.5]                  one STT; sign dies in v = f^2
  P(v) = C0+C1 v+C2 v^2+C3 v^3 ~= cos(2pi f) = cos(x@W[0]), err 1.4e-3
  a = Pp*q3 + Pb = cos(p0)*cos(x@W[0])      Pp = P(v_p) = cos(p0), Pb=Pp*C0,
                                            computed on gpsimd off-path
  out = 0.5 + a*(E0 + E1 u + E2 u^2), u=a^2 degree-2 odd sigmoid on DVE
"""

import math

import numpy as np

_NCORES = 8
_B = 4096
_F = 256
_BS = _B // _NCORES  # 512 samples per core
_NT = _BS // 128     # 4 sample-blocks per partition
_INV_TWO_PI = float(1.0 / (2.0 * math.pi))

# P(v) = C0 + C1 v + C2 v^2 + C3 v^3 ~= cos(2pi f), v = f^2, f in [-.5, .5]
_C0 = 0.9985678609910458
_C1 = -19.552759014070162
_C2 = 61.10740166704636
_C3 = -59.580321884808846
# sigmoid(a) = 0.5 + a*(E0 + E1 u + E2 u^2), u = a^2, a in [-1.01, 1.01]
_E0 = 0.24999587
_E1 = -0.02074685
_E2 = 0.00181964

_CACHE: dict = {}


def _build():
    import concourse.bacc as bacc
    import concourse.mybir as mybir
    import concourse.tile as tile

    f32 = mybir.dt.float32
    i32 = mybir.dt.int32
    Alu = mybir.AluOpType

    nc = bacc.Bacc("TRN2", target_bir_lowering=False, debug=False,
                   num_devices=_NCORES)

    x_d = nc.dram_tensor("x", [_BS, _F], f32, kind="ExternalInput")
    wp_d = nc.dram_tensor("wp", [128, _F + 1], f32, kind="ExternalInput")
    o_d = nc.dram_tensor("o", [_BS], f32, kind="ExternalOutput")

    with tile.TileContext(nc) as tc:
        with (
            tc.tile_pool(name="xin", bufs=1) as xpool,
            tc.tile_pool(name="scratch", bufs=2) as spool,
            tc.tile_pool(name="small", bufs=1) as zpool,
        ):
            # --- input DMAs ---
            wb = zpool.tile([128, _F + 1], f32)
            nc.scalar.dma_start(wb[:], wp_d[:, :])
            xr = x_d.ap().rearrange("(p n) f -> p (n f)", n=_NT)  # [128,1024]
            xt = xpool.tile([128, _NT * _F], f32)
            half = _NT * _F // 2
            nc.gpsimd.dma_start(xt[:, half:], xr[:, half:])
            nc.sync.dma_start(xt[:, 0:half], xr[:, 0:half])

            # --- params chain on gpsimd: Pp = P(frac(p0/2pi)^2) = cos(p0),
            #     Pb = Pp*C0.  [128,1] native ops, off the DVE path. ---
            pz = zpool.tile([128, 1], f32)
            nc.gpsimd.tensor_scalar_mul(pz[:], wb[:, _F:_F + 1], _INV_TWO_PI)
            pk = zpool.tile([128, 1], i32)
            nc.gpsimd.tensor_copy(pk[:], pz[:])
            pkf = zpool.tile([128, 1], f32)
            nc.gpsimd.tensor_copy(pkf[:], pk[:])
            pd = zpool.tile([128, 1], f32)
            nc.gpsimd.tensor_tensor(pd[:], pz[:], pkf[:], op=Alu.subtract)
            pv = zpool.tile([128, 1], f32)
            nc.gpsimd.tensor_tensor(pv[:], pd[:], pd[:], op=Alu.mult)
            ps1 = zpool.tile([128, 1], f32)
            nc.gpsimd.tensor_scalar(ps1[:], pv[:], _C3, _C2,
                                    op0=Alu.mult, op1=Alu.add)
            pm1 = zpool.tile([128, 1], f32)
            nc.gpsimd.tensor_tensor(pm1[:], ps1[:], pv[:], op=Alu.mult)
            ps2 = zpool.tile([128, 1], f32)
            nc.gpsimd.tensor_scalar(ps2[:], pm1[:], _C1, 1.0,
                                    op0=Alu.add, op1=Alu.mult)
            ps3 = zpool.tile([128, 1], f32)
            nc.gpsimd.tensor_tensor(ps3[:], ps2[:], pv[:], op=Alu.mult)
            pp = zpool.tile([128, 1], f32)
            nc.gpsimd.tensor_scalar(pp[:], ps3[:], _C0, 1.0,
                                    op0=Alu.add, op1=Alu.mult)
            pb = zpool.tile([128, 1], f32)
            nc.gpsimd.tensor_scalar(pb[:], ps3[:], _C0, _C0,
                                    op0=Alu.add, op1=Alu.mult)

            # --- dot products z[:, n] = sum_f x_blk_n*inv2pi * w  (DVE) ---
            w256 = wb[:, 0:_F]
            z = zpool.tile([128, _NT], f32)
            for n in range(_NT):
                prod = spool.tile([128, _F], f32)
                nc.vector.scalar_tensor_tensor(
                    prod[:], xt[:, n * _F:(n + 1) * _F], _INV_TWO_PI, w256,
                    op0=Alu.mult, op1=Alu.mult,
                    accum_out=z[:, n:n + 1],
                )

            # --- range reduce + cos poly (DVE): q3 = P(v) - C0 ---
            k = zpool.tile([128, _NT], i32)
            nc.vector.tensor_copy(k[:], z[:])
            kf = zpool.tile([128, _NT], f32)
            nc.vector.tensor_copy(kf[:], k[:])
            f = zpool.tile([128, _NT], f32)
            nc.vector.scalar_tensor_tensor(f[:], kf[:], 0.0, z[:],
                                           op0=Alu.bypass, op1=Alu.subtract)
            v = zpool.tile([128, _NT], f32)
            nc.vector.tensor_tensor(v[:], f[:], f[:], op=Alu.mult)
            q1 = zpool.tile([128, _NT], f32)
            nc.vector.tensor_scalar(q1[:], v[:], _C3, _C2,
                                    op0=Alu.mult, op1=Alu.add)
            q2 = zpool.tile([128, _NT], f32)
            nc.vector.scalar_tensor_tensor(q2[:], q1[:], 0.0, v[:],
                                           op0=Alu.bypass, op1=Alu.mult)
            q3 = zpool.tile([128, _NT], f32)
            nc.vector.scalar_tensor_tensor(q3[:], q2[:], _C1, v[:],
                                           op0=Alu.add, op1=Alu.mult)

            # --- a = Pp*q3 + Pb;  out = 0.5 + a*(E0 + E1 u + E2 u^2) ---
            a = zpool.tile([128, _NT], f32)
            nc.vector.tensor_scalar(a[:], q3[:], pp[:, :], pb[:, :],
                                    op0=Alu.mult, op1=Alu.add)
            u = zpool.tile([128, _NT], f32)
            nc.vector.tensor_tensor(u[:], a[:], a[:], op=Alu.mult)
            h1 = zpool.tile([128, _NT], f32)
            nc.vector.tensor_scalar(h1[:], u[:], _E2, _E1,
                                    op0=Alu.mult, op1=Alu.add)
            h2 = zpool.tile([128, _NT], f32)
            nc.vector.scalar_tensor_tensor(h2[:], h1[:], 0.0, u[:],
                                           op0=Alu.bypass, op1=Alu.mult)
            h3 = zpool.tile([128, _NT], f32)
            nc.vector.scalar_tensor_tensor(h3[:], h2[:], _E0, a[:],
                                           op0=Alu.add, op1=Alu.mult)
            ot = zpool.tile([128, _NT], f32)
            nc.vector.tensor_scalar(ot[:], h3[:], 1.0, 0.5,
                                    op0=Alu.mult, op1=Alu.add)

            # --- output store, split across the two HWDGE rings ---
            orr = o_d.ap().rearrange("(p n) -> p n", n=_NT)
            nc.scalar.dma_start(orr[0:64], ot[0:64, :])
            nc.sync.dma_start(orr[64:128], ot[64:128, :])

    nc.compile()
    return nc


def _get_nc():
    if "nc" not in _CACHE:
        _CACHE["nc"] = _build()
    return _CACHE["nc"]


def _in_maps(x, W, params):
    x = np.ascontiguousarray(np.asarray(x, dtype=np.float32))
    W = np.asarray(W, dtype=np.float32)
    params = np.asarray(params, dtype=np.float32)
    wp_row = np.concatenate([W[0], params[0:1]]).astype(np.float32)
    wp = np.ascontiguousarray(np.broadcast_to(wp_row, (128, _F + 1)))
    return [
        {"x": x[c * _BS:(c + 1) * _BS], "wp": wp}
        for c in range(_NCORES)
    ]


def run_spmd(x, W, params, **kw):
    """Compile (cached) and run on 8 cores; returns BassKernelResults.

    Retries a few times: the axon-relayed device occasionally reports a
    transient NRT_EXEC_UNIT_UNRECOVERABLE that clears on the next attempt.
    """
    import time

    from concourse import bass_utils

    nc = _get_nc()
    in_maps = _in_maps(x, W, params)
    last = None
    for attempt in range(4):
        try:
            return bass_utils.run_bass_kernel_spmd(
                nc, in_maps, list(range(_NCORES)), **kw
            )
        except Exception as e:  # transient device/relay errors
            last = e
            time.sleep(2.0 * (attempt + 1))
    raise last


def kernel(x, W, params):
    res = run_spmd(x, W, params)
    return np.concatenate([res.results[c]["o"] for c in range(_NCORES)], axis=0)
